# revision 4
# baseline (speedup 1.0000x reference)
"""Trainium2 Bass kernel for nn_BiSVM: out[b,o] = diag(L @ x[b] @ R).

Math: out[b,o] = sum_{i,j} L[o,i] * x[b,i,j] * R[j,o]
  step 1 (TensorE): lx[o,j] = sum_i LT[i,o]^T @ x[b,i,j]   (LT = L^T, stationary)
  step 2 (VectorE): out[b,o] = sum_j lx[o,j] * RT[o,j]      (RT = R^T, fused
          multiply+reduce via scalar_tensor_tensor accum_out)

Sharding: data-parallel over batch, 8 batches per core on 8 NeuronCores;
L/R replicated. x and L are cast to fp16 on the host (PE matmul runs fp16 at
full rate, 1 cycle/row; products are exact, accumulation is fp32 in PSUM —
end-to-end error ~3e-4 relative to the fp32 reference). R stays fp32 in the
vector-engine reduction.

Self-contained: hardcodes shapes B=64, I=O=J=1024, 8 cores.
"""

import numpy as np

import concourse.bacc as bacc
import concourse.mybir as mybir
import concourse.tile as tile
from concourse.bass_utils import run_bass_kernel_spmd


def _ldw_sig(i):
    return (str(i.ins[0].bass_ap), str(i.perf_mode), str(i.is_transpose),
            str(i.tile_position))


def _ap_tile_name(ap):
    ba = getattr(ap, "bass_ap", None)
    t = getattr(ba, "tensor", None)
    return getattr(t, "name", None)


def _dedupe_ldweights(ordered):
    """Drop InstLdweights that reload the exact weights already resident in
    the PE array (legalize emits one per matmul; consecutive matmuls that
    share a stationary operand only need the first). Runs before semaphore
    assignment, so sync is computed on the deduped stream. A write (by any
    engine/DMA) to the weight tile between loads invalidates the cache."""
    ndrop = 0
    for bb in list(ordered.keys()):
        out, last_sig, last_tile = [], None, None
        for i in ordered[bb]:
            tn = type(i).__name__
            if getattr(i, "engine", None) == mybir.EngineType.PE:
                if tn == "InstLdweights":
                    sig = _ldw_sig(i)
                    if sig == last_sig:
                        ndrop += 1
                        continue
                    last_sig = sig
                    last_tile = _ap_tile_name(i.ins[0])
                elif tn == "InstMatmult" and not i.is_transpose:
                    pass  # matmuls leave the stationary operand intact
                else:
                    last_sig = last_tile = None
            if last_tile is not None:
                for o in getattr(i, "outs", []):
                    if _ap_tile_name(o) == last_tile:
                        last_sig = last_tile = None
                        break
            out.append(i)
        ordered[bb] = out
    return ndrop


class _patched_legalize:
    def __enter__(self):
        self._orig = tile.tile_legalize

        def wrapper(ordered, nc_, *a, **kw):
            ordered = self._orig(ordered, nc_, *a, **kw)
            n = _dedupe_ldweights(ordered)
            print(f"[kernel] deduped {n} redundant LDWEIGHTS")
            return ordered

        tile.tile_legalize = wrapper
        return self

    def __exit__(self, *exc):
        tile.tile_legalize = self._orig
        return False

B, I, O, J = 64, 1024, 1024, 1024
NCORES = 8
BPC = B // NCORES          # batches per core
BBLK = 2                   # batches per SBUF-resident block
NBLK = BPC // BBLK
NOT = O // 128             # o-tiles
NIT = I // 128             # i-tiles (contraction)
NJC = J // 512             # j-chunks (psum bank width)

f16 = mybir.dt.float16
f32 = mybir.dt.float32


def build_nc(reps: int | None = None):
    nc = bacc.Bacc("TRN2", target_bir_lowering=False, debug=False)
    x_d = nc.dram_tensor("x", [BPC, I, J], f16, kind="ExternalInput")
    lt_d = nc.dram_tensor("lt", [I, O], f16, kind="ExternalInput")
    rt_d = nc.dram_tensor("rt", [O, J], f32, kind="ExternalInput")
    # out_sb layout: [o_within_tile(128), ot(8) * b(8)] ; host reassembles
    out_d = nc.dram_tensor("out", [128, NOT * BPC], f32, kind="ExternalOutput")

    import contextlib

    def body(tc, wpool, xpool, spool, pspool):
            lt_sb = wpool.tile([128, NIT, O], f16, name="lt_sb")

            def load_lt_chunk(lts):
                nc.sync.dma_start(
                    lt_sb[:, lts:lts + 1, :],
                    lt_d.ap()[lts * 128:(lts + 1) * 128, :]
                    .rearrange("(t p) o -> p t o", p=128))

            rt_sb = wpool.tile([128, NOT, J], f32, name="rt_sb")
            out_sb = wpool.tile([128, NOT * BPC], f32, name="out_sb")

            for blk in range(NBLK):
                xts = []
                for bb in range(BBLK):
                    b = blk * BBLK + bb
                    xt = xpool.tile([128, NIT, J], f16,
                                    name=f"x_{b}", tag="xt")
                    for sp in range(NIT):
                        if blk == 0 and bb == 0:
                            load_lt_chunk(sp)
                        nc.sync.dma_start(
                            xt[:, sp:sp + 1, :],
                            x_d.ap()[b, sp * 128:(sp + 1) * 128, :]
                            .rearrange("(t p) j -> p t j", p=128))
                    xts.append(xt)
                if blk == 0:
                    nc.sync.dma_start(
                        rt_sb[:],
                        rt_d.ap().rearrange("(t p) j -> p t j", p=128))
                for ot in range(NOT):
                    pss = [
                        pspool.tile([128, J], f32,
                                    name=f"ps_{blk}_{ot}_{s}", tag="ps")
                        for s in range(BBLK)
                    ]
                    for it in range(NIT):
                        lhsT = lt_sb[:, it, ot * 128:(ot + 1) * 128]
                        for bb in range(BBLK):
                            for jc in range(NJC):
                                nc.tensor.matmul(
                                    pss[bb][:, jc * 512:(jc + 1) * 512],
                                    lhsT,
                                    xts[bb][:, it, jc * 512:(jc + 1) * 512],
                                    start=(it == 0),
                                    stop=(it == NIT - 1),
                                )
                    for bb in range(BBLK):
                        b = blk * BBLK + bb
                        sc0 = spool.tile([128, J], f32,
                                         name=f"sc0_{b}_{ot}", tag="sc")
                        col = ot * BPC + b
                        # out = (ps * 1.0) * rt ; accum_out = sum_j(out)
                        nc.vector.scalar_tensor_tensor(
                            out=sc0[:],
                            in0=pss[bb][:],
                            scalar=1.0,
                            in1=rt_sb[:, ot, :],
                            op0=mybir.AluOpType.mult,
                            op1=mybir.AluOpType.mult,
                            accum_out=out_sb[:, col:col + 1],
                        )
            nc.sync.dma_start(out_d.ap(), out_sb[:])

    with _patched_legalize(), tile.TileContext(nc) as tc:
        with (
            tc.tile_pool(name="w", bufs=1) as wpool,
            tc.tile_pool(name="xp", bufs=2 * BBLK) as xpool,
            tc.tile_pool(name="sc", bufs=4) as spool,
            tc.tile_pool(name="ps", bufs=4, space="PSUM") as pspool,
        ):
            loop = (tc.For_i(0, reps, 1) if reps is not None
                    else contextlib.nullcontext())
            with loop:
                body(tc, wpool, xpool, spool, pspool)
    nc.compile()
    return nc


_NC_CACHE = []


def _get_nc():
    if not _NC_CACHE:
        _NC_CACHE.append(build_nc())
    return _NC_CACHE[0]


def make_in_maps(x: np.ndarray, L: np.ndarray, R: np.ndarray):
    xx = np.ascontiguousarray(x).astype(np.float16)
    lt = np.ascontiguousarray(L.T).astype(np.float16)
    rt = np.ascontiguousarray(R.T).astype(np.float32)
    return [
        {"x": xx[c * BPC:(c + 1) * BPC], "lt": lt, "rt": rt}
        for c in range(NCORES)
    ]


def assemble(results) -> np.ndarray:
    out = np.empty((B, O), np.float32)
    for c in range(NCORES):
        oc = results[c]["out"]                      # [128, NOT*BPC]
        t = oc.reshape(128, NOT, BPC)               # [p, ot, b]
        out[c * BPC:(c + 1) * BPC] = t.transpose(2, 1, 0).reshape(BPC, O)
    return out


def kernel(x: np.ndarray, L: np.ndarray, R: np.ndarray) -> np.ndarray:
    nc = _get_nc()
    res = run_bass_kernel_spmd(nc, make_in_maps(x, L, R),
                               core_ids=list(range(NCORES)))
    return assemble(res.results)



# revision 7
# speedup vs baseline: 1.0006x; 1.0006x over previous
"""Trainium2 Bass kernel for nn_BiSVM: out[b,o] = diag(L @ x[b] @ R).

Math: out[b,o] = sum_{i,j} L[o,i] * x[b,i,j] * R[j,o]
  step 1 (TensorE): lx[o,j] = sum_i LT[i,o]^T @ x[b,i,j]   (LT = L^T, stationary)
  step 2 (VectorE): out[b,o] = sum_j lx[o,j] * RT[o,j]      (RT = R^T, fused
          multiply+reduce via scalar_tensor_tensor accum_out)

Sharding: data-parallel over batch, 8 batches per core on 8 NeuronCores;
L/R replicated. x and L are cast to fp16 on the host (PE matmul runs fp16 at
full rate, 1 cycle/row; products are exact, accumulation is fp32 in PSUM —
end-to-end error ~3e-4 relative to the fp32 reference). R stays fp32 in the
vector-engine reduction.

Self-contained: hardcodes shapes B=64, I=O=J=1024, 8 cores.
"""

import numpy as np
from collections import defaultdict

import concourse.bacc as bacc
import concourse.mybir as mybir
import concourse.tile as tile
from concourse.bass_utils import run_bass_kernel_spmd


def _ldw_sig(i):
    return (str(i.ins[0].bass_ap), str(i.perf_mode), str(i.is_transpose),
            str(i.tile_position))


def _ap_tile_name(ap):
    ba = getattr(ap, "bass_ap", None)
    t = getattr(ba, "tensor", None)
    return getattr(t, "name", None)


def _dedupe_ldweights(ordered):
    """Drop InstLdweights that reload the exact weights already resident in
    the PE array (legalize emits one per matmul; consecutive matmuls that
    share a stationary operand only need the first). Runs before semaphore
    assignment, so sync is computed on the deduped stream. A write (by any
    engine/DMA) to the weight tile between loads invalidates the cache."""
    ndrop = 0
    for bb in list(ordered.keys()):
        out, last_sig, last_tile = [], None, None
        for i in ordered[bb]:
            tn = type(i).__name__
            if getattr(i, "engine", None) == mybir.EngineType.PE:
                if tn == "InstLdweights":
                    sig = _ldw_sig(i)
                    if sig == last_sig:
                        ndrop += 1
                        continue
                    last_sig = sig
                    last_tile = _ap_tile_name(i.ins[0])
                elif tn == "InstMatmult" and not i.is_transpose:
                    pass  # matmuls leave the stationary operand intact
                else:
                    last_sig = last_tile = None
            if last_tile is not None:
                for o in getattr(i, "outs", []):
                    if _ap_tile_name(o) == last_tile:
                        last_sig = last_tile = None
                        break
            out.append(i)
        ordered[bb] = out
    return ndrop


def _thin_sem_incs(nc, min_updates=256):
    """Drop per-matmul semaphore increments that no wait ever lands on
    (engines pay ~26 ns per EVT_SEM write; a kernel with 1024 matmuls carries
    1024 incs but only ~64 waited values). Keeps exactly the inc whose
    cumulative value first satisfies each waited value (so every wait fires at
    the same instruction-completion as before), then renumbers all waits on
    that semaphore into the compressed counting (each kept inc stays +1)."""
    fn = nc.m.functions[0]
    blocks = list(fn.blocks)
    # Pass 1: waited values per sem (across all blocks) + bail-out flags.
    waited = {}
    bad = set()
    for b in blocks:
        for i in b.instructions:
            si = i.sync_info
            if si is None:
                continue
            for w in si.on_wait:
                if w.ant_name is None:
                    continue
                if w.wait_mode != "sem-ge-imm" or w.wait_reg is not None:
                    bad.add(w.ant_name)
                else:
                    waited.setdefault(w.ant_name, set()).add(w.wait_value)
            for u in si.on_update:
                if u.ant_name is None:
                    continue
                if u.update_mode != "sem-inc" or u.update_reg is not None:
                    bad.add(u.ant_name)
    # Pass 2: per sem, decide kept incs + wait renumber map. Incs for one sem
    # must live in a single block (loop bodies would break cross-block
    # counting) — enforce, else bail.
    inc_block = {}
    incs = defaultdict(list)  # sem -> [(inst, upd, cum_after)]
    cum = defaultdict(int)
    for b in blocks:
        for i in b.instructions:
            si = i.sync_info
            if si is None:
                continue
            for u in si.on_update:
                s = u.ant_name
                if s is None or s in bad:
                    continue
                if s in inc_block and inc_block[s] != b.name:
                    bad.add(s)
                    continue
                inc_block[s] = b.name
                cum[s] += u.update_value
                incs[s].append((i, u, cum[s]))
    ndrop = 0
    for sem, lst in incs.items():
        if sem in bad or len(lst) < min_updates:
            continue
        wvals = sorted(waited.get(sem, set()))
        keep = [False] * len(lst)
        keep[-1] = True
        it = iter(wvals)
        nxt = next(it, None)
        for k, (_, _, c) in enumerate(lst):
            while nxt is not None and c >= nxt:
                keep[k] = True
                nxt = next(it, None)
        kept_cums = [c for k, (_, _, c) in enumerate(lst) if keep[k]]
        # new wait value for original v = rank (1-based) of first kept cum >= v
        import bisect
        def new_wait(v, kc=kept_cums):
            return bisect.bisect_left(kc, v) + 1
        # drop unkept incs (each kept inc remains +1)
        for k, (inst, u, _) in enumerate(lst):
            if keep[k]:
                continue
            si = inst.sync_info
            ups = [x for x in si.on_update if x is not u]
            inst.sync_info = mybir.SyncInfo(
                on_wait=list(si.on_wait), on_update=ups)
            ndrop += 1
        # renumber every wait on this sem, in every block
        for b in blocks:
            for i in b.instructions:
                si = i.sync_info
                if si is None:
                    continue
                if not any(w.ant_name == sem for w in si.on_wait):
                    continue
                new_ws = []
                for w in si.on_wait:
                    if w.ant_name == sem:
                        new_ws.append(mybir.SyncWait(
                            sync_type=w.sync_type, id=w.id, ant_name=w.ant_name,
                            wait_mode=w.wait_mode,
                            wait_value=new_wait(w.wait_value)))
                    else:
                        new_ws.append(w)
                i.sync_info = mybir.SyncInfo(
                    on_wait=new_ws, on_update=list(i.sync_info.on_update))
    return ndrop


class _patched_legalize:
    def __enter__(self):
        self._orig = tile.tile_legalize

        def wrapper(ordered, nc_, *a, **kw):
            ordered = self._orig(ordered, nc_, *a, **kw)
            n = _dedupe_ldweights(ordered)
            print(f"[kernel] deduped {n} redundant LDWEIGHTS")
            return ordered

        tile.tile_legalize = wrapper
        return self

    def __exit__(self, *exc):
        tile.tile_legalize = self._orig
        return False

B, I, O, J = 64, 1024, 1024, 1024
NCORES = 8
BPC = B // NCORES          # batches per core
BBLK = 2                   # batches per SBUF-resident block
NBLK = BPC // BBLK
NOT = O // 128             # o-tiles
NIT = I // 128             # i-tiles (contraction)
NJC = J // 512             # j-chunks (psum bank width)

f16 = mybir.dt.float16
f32 = mybir.dt.float32


def build_nc(reps: int | None = None):
    nc = bacc.Bacc("TRN2", target_bir_lowering=False, debug=False)
    x_d = nc.dram_tensor("x", [BPC, I, J], f16, kind="ExternalInput")
    lt_d = nc.dram_tensor("lt", [I, O], f16, kind="ExternalInput")
    rt_d = nc.dram_tensor("rt", [O, J], f32, kind="ExternalInput")
    # out_sb layout: [o_within_tile(128), ot(8) * b(8)] ; host reassembles
    out_d = nc.dram_tensor("out", [128, NOT * BPC], f32, kind="ExternalOutput")

    import contextlib

    def body(tc, wpool, xpool, spool, pspool):
            lt_sb = wpool.tile([128, NIT, O], f16, name="lt_sb")

            def load_lt_chunk(lts):
                nc.sync.dma_start(
                    lt_sb[:, lts:lts + 1, :],
                    lt_d.ap()[lts * 128:(lts + 1) * 128, :]
                    .rearrange("(t p) o -> p t o", p=128))

            rt_sb = wpool.tile([128, NOT, J], f32, name="rt_sb")
            out_sb = wpool.tile([128, NOT * BPC], f32, name="out_sb")

            for blk in range(NBLK):
                xts = []
                for bb in range(BBLK):
                    b = blk * BBLK + bb
                    xt = xpool.tile([128, NIT, J], f16,
                                    name=f"x_{b}", tag="xt")
                    for sp in range(NIT):
                        if blk == 0 and bb == 0:
                            load_lt_chunk(sp)
                        nc.sync.dma_start(
                            xt[:, sp:sp + 1, :],
                            x_d.ap()[b, sp * 128:(sp + 1) * 128, :]
                            .rearrange("(t p) j -> p t j", p=128))
                    xts.append(xt)
                if blk == 0:
                    nc.sync.dma_start(
                        rt_sb[:],
                        rt_d.ap().rearrange("(t p) j -> p t j", p=128))
                for ot in range(NOT):
                    pss = [
                        pspool.tile([128, J], f32,
                                    name=f"ps_{blk}_{ot}_{s}", tag="ps")
                        for s in range(BBLK)
                    ]
                    for it in range(NIT):
                        lhsT = lt_sb[:, it, ot * 128:(ot + 1) * 128]
                        for bb in range(BBLK):
                            for jc in range(NJC):
                                nc.tensor.matmul(
                                    pss[bb][:, jc * 512:(jc + 1) * 512],
                                    lhsT,
                                    xts[bb][:, it, jc * 512:(jc + 1) * 512],
                                    start=(it == 0),
                                    stop=(it == NIT - 1),
                                )
                    for bb in range(BBLK):
                        b = blk * BBLK + bb
                        sc0 = spool.tile([128, J], f32,
                                         name=f"sc0_{b}_{ot}", tag="sc")
                        col = ot * BPC + b
                        # out = (ps * 1.0) * rt ; accum_out = sum_j(out)
                        nc.vector.scalar_tensor_tensor(
                            out=sc0[:],
                            in0=pss[bb][:],
                            scalar=1.0,
                            in1=rt_sb[:, ot, :],
                            op0=mybir.AluOpType.mult,
                            op1=mybir.AluOpType.mult,
                            accum_out=out_sb[:, col:col + 1],
                        )
            nc.sync.dma_start(out_d.ap(), out_sb[:])

    with _patched_legalize(), tile.TileContext(nc) as tc:
        with (
            tc.tile_pool(name="w", bufs=1) as wpool,
            tc.tile_pool(name="xp", bufs=2 * BBLK) as xpool,
            tc.tile_pool(name="sc", bufs=4) as spool,
            tc.tile_pool(name="ps", bufs=4, space="PSUM") as pspool,
        ):
            loop = (tc.For_i(0, reps, 1) if reps is not None
                    else contextlib.nullcontext())
            with loop:
                body(tc, wpool, xpool, spool, pspool)
    nc.compile()
    n = _thin_sem_incs(nc)
    print(f"[kernel] thinned {n} sem increments")
    return nc


_NC_CACHE = []


def _get_nc():
    if not _NC_CACHE:
        _NC_CACHE.append(build_nc())
    return _NC_CACHE[0]


def make_in_maps(x: np.ndarray, L: np.ndarray, R: np.ndarray):
    xx = np.ascontiguousarray(x).astype(np.float16)
    lt = np.ascontiguousarray(L.T).astype(np.float16)
    rt = np.ascontiguousarray(R.T).astype(np.float32)
    return [
        {"x": xx[c * BPC:(c + 1) * BPC], "lt": lt, "rt": rt}
        for c in range(NCORES)
    ]


def assemble(results) -> np.ndarray:
    out = np.empty((B, O), np.float32)
    for c in range(NCORES):
        oc = results[c]["out"]                      # [128, NOT*BPC]
        t = oc.reshape(128, NOT, BPC)               # [p, ot, b]
        out[c * BPC:(c + 1) * BPC] = t.transpose(2, 1, 0).reshape(BPC, O)
    return out


def kernel(x: np.ndarray, L: np.ndarray, R: np.ndarray) -> np.ndarray:
    nc = _get_nc()
    res = run_bass_kernel_spmd(nc, make_in_maps(x, L, R),
                               core_ids=list(range(NCORES)))
    return assemble(res.results)

